# revision 54
# baseline (speedup 1.0000x reference)
"""Bahdanau additive attention on 8 TRN2 NeuronCores (batch-parallel).

Math: scores[b,i,j] = q[b,i].w + k[b,j].w, masked to -1e9 where mask==0,
softmax over j, then @ value.  The query term q[b,i].w is constant along j,
so it cancels in the softmax:

    out[b,i,:] = (sum_j mask[b,i,j] * e[b,j] * value[b,j,:])
               / (sum_j mask[b,i,j] * e[b,j]),      e[b,j] = exp(k[b,j].w)

(no query needed, no [Lq,Lk] softmax).  Per core: one batch.

Layout strategy: the PE contracts over partitions, so the mask needs j on
partitions.  Rather than transposing on-chip (256 PE transposes ~ 27us),
the host uploads the mask PRE-TRANSPOSED as uint8 in j-major tile order:
maskt[p, s, t*128+c] = mask[i=128t+c, j=128s+p].  That's 4x fewer HBM
bytes than int32 and removes all PE transpose work.  The 0/1 bytes become
fp16 0.0/1.0 stationary operands via three parallel converters that depend
ONLY on the mask bytes (not on the exp chain):
  - SWDGE cast-DMA (u8 -> f16 during the DMA itself, gpsimd ring)
  - DVE tensor_scalar is_gt (u8 in, f16 out)
  - ACT activation-copy (u8 in, f16 out)
The matmul accumulates psum[i, 0:257] = sum_j maskT[j,i] * [e*v | e][j,:]
over 16 j-strips; col 256 gives the softmax denominator.  16 i-tiles run
in two waves of 8 psum banks; epilogue divides and stores fp16, upcast on
the host.

DMA orchestration (the performance-critical part): the 16 SDMA engines
serve all rings round-robin at packet granularity, so a transfer's
completion time tracks the TOTAL dispatched backlog, not its own size;
within one HWDGE ring completions are FIFO.  So: keep total bytes low
(k/wrep in fp16, packed with v into one tensor = 2.1MB; mask u8 4.2MB;
only 4 strips take the 2x-write SWDGE cast path), dispatch in consumption
order per ring, and hold the SWDGE ring back with a dummy gpsimd memset so
the critical kv head isn't diluted at kernel start.  Each [128 x N] HWDGE
dispatch also costs ~0.65us descriptor-generation on its ring, so DMA
count per ring is kept small.

The Tile scheduler's internal DMA-cost model is far too optimistic; left
alone it bakes head-of-line blocking into the engine FIFOs (an op whose
data lands at 22us ordered ahead of ops ready at 14us).  tile_wait_until
annotations carry measured arrival times into the scheduling simulation.

A dependency-free burst of dummy matmuls at kernel start trips the PE HAM
activity monitor to full clock before real work arrives.
"""

import os
import sys
import types

sys.path.insert(0, "/opt/trn_rl_repo")

import numpy as np

import concourse.bacc as bacc
import concourse.tile as tile
from concourse import mybir
from concourse.bass_utils import run_bass_kernel_spmd


def _ensure_ntff_hook_importable():
    """bass_utils imports antenv.axon_hooks when BASS_TRACE is set; this
    image's antenv lacks that module.  Provide it (and register the real
    ctypes NTFF hook if available) so tracing works instead of crashing."""
    if "antenv.axon_hooks" in sys.modules:
        return
    try:
        import antenv
    except ImportError:
        return
    hooks = types.ModuleType("antenv.axon_hooks")
    hooks._hook = None
    hooks.set_axon_ntff_profile_hook = lambda h: setattr(hooks, "_hook", h)
    hooks.get_axon_ntff_profile_hook = lambda: hooks._hook
    sys.modules["antenv.axon_hooks"] = hooks
    antenv.axon_hooks = hooks
    try:
        from trn_agent_boot.trn_boot import _ntff_profile_via_ctypes

        hook = _ntff_profile_via_ctypes("/opt/axon/libaxon_pjrt.so")
        if hook is not None:
            hooks.set_axon_ntff_profile_hook(hook)
    except Exception:
        pass


_ensure_ntff_hook_importable()

P = 128
B = 8
L = 2048
D = 256
NT = L // P  # 16 tiles per dim
NE = D + 1  # 257 = value cols + e col (matmul moving width)
VP = D + 2  # 258 = ev row pitch (even, for engine perf modes)

# packed wrep/k/v record geometry, in fp16 elements per partition
KV_WREP = D  # wrep: 256 f16
KV_REC = D + VP  # per strip: k 256 f16 + v 258 f16
KV_TOT = KV_WREP + NT * KV_REC

# strip -> converter assignment (tunable).  The tail strips are SWDGE
# casts so their readiness is the DMA arrival itself -- no converter-queue
# lag at the end of the supply stream, where it directly sets wave A's end.
CAST_STRIPS = (0, 2, 5, 10, 13, 14)  # SWDGE u8->f16 cast-DMA
DVE_STRIPS = (1, 3, 4, 7, 9, 12, 15)  # u8 load + DVE scaled cast
ACT_STRIPS = (6, 8, 11)  # u8 load + ACT scaled copy
U8_GROUPS = ((1,), (3, 4), (6, 7), (8, 9), (15,))  # scalar ring
U8_SYNC_GROUP = (11, 12)  # rides the sync ring right behind the kv chunks
N_WARM = 9

# scheduler hints: realistic data-arrival times (ms) for tile_wait_until,
# measured from HW traces of this exact configuration.
KV_ARRIVE = (0.012, 0.018, 0.023, 0.028)
U8_ARRIVE = {1: 0.0105, 2: 0.013, 3: 0.0165, 4: 0.0165, 6: 0.021, 7: 0.021,
             8: 0.026, 9: 0.026, 11: 0.0295, 12: 0.0295, 13: 0.028,
             14: 0.028, 15: 0.031}

LAST_RESULTS = None


def _build_nc():
    dt = mybir.dt
    nc = bacc.Bacc("TRN2", target_bir_lowering=False, debug=False, num_devices=B)

    maskt_d = nc.dram_tensor("maskt", [P, NT * L], dt.uint8, kind="ExternalInput").ap()
    kv_d = nc.dram_tensor("kv", [P, KV_TOT], dt.float16, kind="ExternalInput").ap()
    out_d = nc.dram_tensor("out", [P, NT * D], dt.float16, kind="ExternalOutput").ap()

    with tile.TileContext(nc) as tc:
        with (
            tc.tile_pool(name="const", bufs=1) as const_pool,
            tc.tile_pool(name="kv", bufs=1) as kv_pool,
            tc.tile_pool(name="small", bufs=1) as small_pool,
            tc.tile_pool(name="junk", bufs=2) as junk_pool,
            tc.tile_pool(name="mu8", bufs=7) as mu8_pool,
            tc.tile_pool(name="outp", bufs=2) as out_pool,
            tc.tile_pool(name="rec", bufs=4) as rec_pool,
            tc.tile_pool(name="acc", bufs=8, space="PSUM") as acc_pool,
        ):
            # HAM warmup: dummy matmuls with no real dependencies (zeroed
            # data; results never read) to bring the PE to full clock.
            # memset on gpsimd: the vector queue's preamble is longer.
            warm_mv = const_pool.tile([P, 512], dt.float16)
            nc.gpsimd.memset(warm_mv[:], 0.0)
            warm_ps = acc_pool.tile([P, 512], dt.float32, tag="acc", name="warm")
            for _ in range(N_WARM):
                nc.tensor.matmul(
                    warm_ps[:], warm_mv[:, 0:P], warm_mv[:], start=True, stop=True
                )

            kv_sb = kv_pool.tile([P, KV_TOT], dt.float16, tag="kvsb")
            wrep = kv_sb[:, 0:KV_WREP]

            def k_ap(s):
                o = KV_WREP + s * KV_REC
                return kv_sb[:, o : o + D]

            def v_ap(s):
                o = KV_WREP + s * KV_REC + D
                return kv_sb[:, o : o + VP]

            # sync ring: packed kv chunks in strip order (head = wrep+s0-3)
            edges = [0] + [KV_WREP + 4 * (c + 1) * KV_REC for c in range(4)]
            for c in range(4):
                sl = slice(edges[c], edges[c + 1])
                nc.sync.dma_start(kv_sb[:, sl], kv_d[:, sl])

            # second warm burst, gated on the kv head DMA via its operands:
            # bridges the PE-idle gap between the dep-free burst and the
            # first real matmul so the HAM activity window never rethrottles
            for _ in range(10):
                nc.tensor.matmul(
                    warm_ps[:], kv_sb[:, 0:P], kv_sb[:, 0:512], start=True, stop=True
                )

            mask16 = kv_pool.tile([P, NT * L], dt.float16, tag="m16")
            m16v = mask16[:].rearrange("p (s i) -> p s i", s=NT)

            # u8 strip groups: scalar ring in consumption order; the
            # (11,12) group rides the sync ring right behind the kv chunks
            # (lands ~29.6us vs ~31+ as the scalar ring's last group),
            # balancing the three rings' supply tails.
            mu8 = {}

            def _load_u8_grp(engine, g):
                a, bb = g[0], g[-1]
                t8 = mu8_pool.tile(
                    [P, (bb - a + 1) * L], dt.uint8, tag="mu8", name=f"mu8_{a}"
                )
                engine.dma_start(t8[:], maskt_d[:, a * L : (bb + 1) * L])
                for s in g:
                    mu8[s] = (t8, (s - a) * L)

            with tc.tile_wait_until(0.0135):
                _load_u8_grp(nc.sync, U8_SYNC_GROUP)
            for g in U8_GROUPS:
                a, bb = g[0], g[-1]
                t8 = mu8_pool.tile(
                    [P, (bb - a + 1) * L], dt.uint8, tag="mu8", name=f"mu8_{a}"
                )
                nc.scalar.dma_start(t8[:], maskt_d[:, a * L : (bb + 1) * L])
                for s in g:
                    mu8[s] = (t8, (s - a) * L)

            # gpsimd ring: first cast strip immediately; delay the rest with
            # a dummy memset so the kv head isn't diluted at kernel start.
            def cast(s):
                sl = slice(s * L, (s + 1) * L)
                nc.gpsimd.dma_start(mask16[:, sl], maskt_d[:, sl])

            cast(CAST_STRIPS[0])
            delay = const_pool.tile([P, 1024], dt.float32)
            nc.gpsimd.memset(delay[:], 0.0)
            for s in CAST_STRIPS[1:]:
                cast(s)

            # ---- prologue per chunk of 4 strips: sk = k.w ; e = exp(sk) ;
            # ev rows [e*v | e]; converters are independent of this chain.
            sk = small_pool.tile([P, NT], dt.float32, tag="sk")
            e_sb = small_pool.tile([P, NT], dt.float32, tag="e")
            ev = kv_pool.tile([P, NT * VP], dt.float16, tag="ev")
            ev3 = ev[:].rearrange("p (s n) -> p s n", n=VP)

            for c in range(4):
                with tc.tile_wait_until(KV_ARRIVE[c]), tc.high_priority():
                    for s in range(4 * c, 4 * c + 4):
                        junk = junk_pool.tile([P, D], dt.float16, tag="junk")
                        nc.vector.scalar_tensor_tensor(
                            out=junk[:],
                            in0=k_ap(s),
                            scalar=1.0,
                            in1=wrep,
                            op0=mybir.AluOpType.mult,
                            op1=mybir.AluOpType.mult,
                            accum_out=sk[:, s : s + 1],
                        )
                    cs = slice(4 * c, 4 * c + 4)
                    nc.scalar.activation(
                        e_sb[:, cs], sk[:, cs], mybir.ActivationFunctionType.Exp
                    )
                    # scaled moving row [e*v | e] only for this chunk's cast
                    # strip; u8 strips get e fused into the stationary and
                    # use the host-packed [v | 1] moving directly.
                    for s in range(4 * c, 4 * c + 4):
                        if s in CAST_STRIPS:
                            nc.vector.tensor_scalar_mul(
                                ev3[:, s, 0:D], v_ap(s)[:, 0:D], e_sb[:, s : s + 1]
                            )
                            nc.vector.tensor_copy(
                                ev3[:, s : s + 1, D], e_sb[:, s : s + 1]
                            )
                # mask conversions (cast-to-f16 fused with the e_j scale)
                for s in range(4 * c, 4 * c + 4):
                    if s in CAST_STRIPS:
                        continue
                    t8, off = mu8[s]
                    src = t8[:, off : off + L]
                    with tc.tile_wait_until(max(U8_ARRIVE[s], KV_ARRIVE[c] + 0.0015)):
                        if s in DVE_STRIPS:
                            nc.vector.tensor_scalar_mul(
                                mask16[:, s * L : (s + 1) * L], src, e_sb[:, s : s + 1]
                            )
                        else:
                            nc.scalar.mul(
                                mask16[:, s * L : (s + 1) * L], src, e_sb[:, s : s + 1]
                            )

            # ---- two waves of 8 i-tiles; 16 accumulating matmuls each.
            # Wave epilogue (reciprocal + scale + f16 stage) is interleaved
            # with the final matmuls; results stream out per tile-pair on
            # the sync ring during the remaining matmuls.
            for w in range(2):
                accs = []
                for t in range(8 * w, 8 * w + 8):
                    accs.append(
                        acc_pool.tile([P, NE], dt.float32, tag="acc", name=f"acc{t}")
                    )
                outb = out_pool.tile([P, 8 * D], dt.float16, tag="outb", name=f"outb{w}")
                for s in range(NT):
                    mov = ev3[:, s, 0:NE] if s in CAST_STRIPS else v_ap(s)[:, 0:NE]
                    for ti, t in enumerate(range(8 * w, 8 * w + 8)):
                        nc.tensor.matmul(
                            accs[ti][:],
                            m16v[:, s, t * P : (t + 1) * P],
                            mov,
                            start=(s == 0),
                            stop=(s == NT - 1),
                        )
                    # zero-matmuls (+0 into an accumulator, exact no-op)
                    # at supply-stall-prone strip boundaries of wave A:
                    # they execute from the PE FIFO during the stall and
                    # keep the HAM activity window busy, so the stream
                    # resumes at 2.4 GHz instead of re-warming from 1.2
                    if w == 0 and 1 <= s <= 12:
                        for _ in range(3):
                            nc.tensor.matmul(
                                accs[7][:],
                                warm_mv[:, 0:P],
                                warm_mv[:, 0:NE],
                                start=False,
                                stop=False,
                            )
                    for ti, t in enumerate(range(8 * w, 8 * w + 8)):
                        if s == NT - 1:
                            acc = accs[ti]
                            rec = rec_pool.tile([P, 1], dt.float32, tag="rec")
                            nc.vector.reciprocal(rec[:], acc[:, D : D + 1])
                            # alternate the scale between ACT and DVE so
                            # tile pairs drain in parallel (frees psum banks
                            # for the next wave sooner, shortens the tail)
                            if ti % 2 == 0:
                                nc.scalar.mul(
                                    outb[:, ti * D : (ti + 1) * D], acc[:, 0:D], rec[:]
                                )
                            else:
                                nc.vector.tensor_scalar_mul(
                                    outb[:, ti * D : (ti + 1) * D], acc[:, 0:D], rec[:]
                                )
                                # alternate store rings so the final pair
                                # stores drain in parallel, not ring-FIFO
                                eng = nc.sync if (ti // 2) % 2 == 0 else nc.scalar
                                eng.dma_start(
                                    out_d[:, (8 * w + ti - 1) * D : (8 * w + ti + 1) * D],
                                    outb[:, (ti - 1) * D : (ti + 1) * D],
                                )

    nc.compile()
    return nc


def kernel(query, key, value, mask, w_align):
    global LAST_RESULTS
    key = np.asarray(key, dtype=np.float32)
    value = np.asarray(value, dtype=np.float32)
    mask = np.asarray(mask)
    w_align = np.asarray(w_align, dtype=np.float32)

    nc = _build_nc()
    in_maps = []
    for b in range(B):
        # maskt[p, s, t*128+c] = mask[b][i=128t+c, j=128s+p]
        mt = (
            mask[b]
            .astype(np.uint8)
            .reshape(NT, P, NT, P)  # [t, c, s, p]
            .transpose(3, 2, 0, 1)  # [p, s, t, c]
            .reshape(P, NT * L)
        )
        # packed wrep/k/v records, all fp16
        kvb = np.zeros((P, KV_TOT), dtype=np.float16)
        kvb[:, 0:KV_WREP] = w_align[None, :]
        kb = key[b].reshape(NT, P, D).transpose(1, 0, 2)  # [p, s, d]
        vb = value[b].reshape(NT, P, D).transpose(1, 0, 2)
        for s in range(NT):
            o = KV_WREP + s * KV_REC
            kvb[:, o : o + D] = kb[:, s]
            kvb[:, o + D : o + D + D] = vb[:, s]
            kvb[:, o + D + D] = 1.0  # ones col (denominator path headroom)
        in_maps.append({"maskt": np.ascontiguousarray(mt), "kv": kvb})
    try:
        res = run_bass_kernel_spmd(nc, in_maps, core_ids=list(range(B)))
    except Exception:
        # e.g. trace requested but profiling unavailable -- retry untraced
        os.environ["BASS_NEVER_TRACE"] = "1"
        res = run_bass_kernel_spmd(nc, in_maps, core_ids=list(range(B)))
    LAST_RESULTS = res
    out = np.empty((B, L, D), dtype=np.float32)
    for b in range(B):
        ob = res.results[b]["out"].astype(np.float32)  # [p, t*D]
        out[b] = ob.reshape(P, NT, D).transpose(1, 0, 2).reshape(L, D)
    return out


# revision 55
# speedup vs baseline: 1.0640x; 1.0640x over previous
"""Bahdanau additive attention on 8 TRN2 NeuronCores (batch-parallel).

Math: scores[b,i,j] = q[b,i].w + k[b,j].w, masked to -1e9 where mask==0,
softmax over j, then @ value.  The query term q[b,i].w is constant along j,
so it cancels in the softmax:

    out[b,i,:] = (sum_j mask[b,i,j] * e[b,j] * value[b,j,:])
               / (sum_j mask[b,i,j] * e[b,j]),      e[b,j] = exp(k[b,j].w)

(no query needed, no [Lq,Lk] softmax).  Per core: one batch.

Layout strategy: the PE contracts over partitions, so the mask needs j on
partitions.  Rather than transposing on-chip (256 PE transposes ~ 27us),
the host uploads the mask PRE-TRANSPOSED as uint8 in j-major tile order:
maskt[p, s, t*128+c] = mask[i=128t+c, j=128s+p].  That's 4x fewer HBM
bytes than int32 and removes all PE transpose work.  The 0/1 bytes become
fp16 0.0/1.0 stationary operands via three parallel converters that depend
ONLY on the mask bytes (not on the exp chain):
  - SWDGE cast-DMA (u8 -> f16 during the DMA itself, gpsimd ring)
  - DVE tensor_scalar is_gt (u8 in, f16 out)
  - ACT activation-copy (u8 in, f16 out)
The matmul accumulates psum[i, 0:257] = sum_j maskT[j,i] * [e*v | e][j,:]
over 16 j-strips; col 256 gives the softmax denominator.  16 i-tiles run
in two waves of 8 psum banks; epilogue divides and stores fp16, upcast on
the host.

DMA orchestration (the performance-critical part): the 16 SDMA engines
serve all rings round-robin at packet granularity, so a transfer's
completion time tracks the TOTAL dispatched backlog, not its own size;
within one HWDGE ring completions are FIFO.  So: keep total bytes low
(k/wrep in fp16, packed with v into one tensor = 2.1MB; mask u8 4.2MB;
only 4 strips take the 2x-write SWDGE cast path), dispatch in consumption
order per ring, and hold the SWDGE ring back with a dummy gpsimd memset so
the critical kv head isn't diluted at kernel start.  Each [128 x N] HWDGE
dispatch also costs ~0.65us descriptor-generation on its ring, so DMA
count per ring is kept small.

The Tile scheduler's internal DMA-cost model is far too optimistic; left
alone it bakes head-of-line blocking into the engine FIFOs (an op whose
data lands at 22us ordered ahead of ops ready at 14us).  tile_wait_until
annotations carry measured arrival times into the scheduling simulation.

A dependency-free burst of dummy matmuls at kernel start trips the PE HAM
activity monitor to full clock before real work arrives.
"""

import os
import sys
import types

sys.path.insert(0, "/opt/trn_rl_repo")

import numpy as np

import concourse.bacc as bacc
import concourse.tile as tile
from concourse import mybir
from concourse.bass_utils import run_bass_kernel_spmd


def _ensure_ntff_hook_importable():
    """bass_utils imports antenv.axon_hooks when BASS_TRACE is set; this
    image's antenv lacks that module.  Provide it (and register the real
    ctypes NTFF hook if available) so tracing works instead of crashing."""
    if "antenv.axon_hooks" in sys.modules:
        return
    try:
        import antenv
    except ImportError:
        return
    hooks = types.ModuleType("antenv.axon_hooks")
    hooks._hook = None
    hooks.set_axon_ntff_profile_hook = lambda h: setattr(hooks, "_hook", h)
    hooks.get_axon_ntff_profile_hook = lambda: hooks._hook
    sys.modules["antenv.axon_hooks"] = hooks
    antenv.axon_hooks = hooks
    try:
        from trn_agent_boot.trn_boot import _ntff_profile_via_ctypes

        hook = _ntff_profile_via_ctypes("/opt/axon/libaxon_pjrt.so")
        if hook is not None:
            hooks.set_axon_ntff_profile_hook(hook)
    except Exception:
        pass


_ensure_ntff_hook_importable()

P = 128
B = 8
L = 2048
D = 256
NT = L // P  # 16 tiles per dim
NE = D + 1  # 257 = value cols + e col (matmul moving width)
VP = D + 2  # 258 = ev row pitch (even, for engine perf modes)

# packed wrep/k/v record geometry, in fp16 elements per partition
KV_WREP = D  # wrep: 256 f16
KV_REC = D + VP  # per strip: k 256 f16 + v 258 f16
KV_TOT = KV_WREP + NT * KV_REC

# strip -> converter assignment (tunable).  The tail strips are SWDGE
# casts so their readiness is the DMA arrival itself -- no converter-queue
# lag at the end of the supply stream, where it directly sets wave A's end.
CAST_STRIPS = (0, 2, 5, 10, 13, 14, 15)  # SWDGE u8->f16 cast-DMA
DVE_STRIPS = (1, 3, 4, 7, 9, 12)  # u8 load + DVE scaled cast
ACT_STRIPS = (6, 8, 11)  # u8 load + ACT scaled copy
U8_GROUPS = ((1,), (3, 4), (6, 7), (8, 9), (11, 12))
N_WARM = 9

# scheduler hints: realistic data-arrival times (ms) for tile_wait_until,
# measured from HW traces of this exact configuration.
KV_ARRIVE = (0.012, 0.018, 0.023, 0.028)
U8_ARRIVE = {1: 0.0105, 2: 0.013, 3: 0.0165, 4: 0.0165, 6: 0.021, 7: 0.021,
             8: 0.026, 9: 0.026, 11: 0.030, 12: 0.030, 13: 0.028, 14: 0.028}

LAST_RESULTS = None


def _build_nc():
    dt = mybir.dt
    nc = bacc.Bacc("TRN2", target_bir_lowering=False, debug=False, num_devices=B)

    maskt_d = nc.dram_tensor("maskt", [P, NT * L], dt.uint8, kind="ExternalInput").ap()
    kv_d = nc.dram_tensor("kv", [P, KV_TOT], dt.float16, kind="ExternalInput").ap()
    out_d = nc.dram_tensor("out", [P, NT * D], dt.float16, kind="ExternalOutput").ap()

    with tile.TileContext(nc) as tc:
        with (
            tc.tile_pool(name="const", bufs=1) as const_pool,
            tc.tile_pool(name="kv", bufs=1) as kv_pool,
            tc.tile_pool(name="small", bufs=1) as small_pool,
            tc.tile_pool(name="junk", bufs=2) as junk_pool,
            tc.tile_pool(name="mu8", bufs=7) as mu8_pool,
            tc.tile_pool(name="outp", bufs=2) as out_pool,
            tc.tile_pool(name="rec", bufs=4) as rec_pool,
            tc.tile_pool(name="acc", bufs=8, space="PSUM") as acc_pool,
        ):
            # HAM warmup: dummy matmuls with no real dependencies (zeroed
            # data; results never read) to bring the PE to full clock.
            # memset on gpsimd: the vector queue's preamble is longer.
            warm_mv = const_pool.tile([P, 512], dt.float16)
            nc.gpsimd.memset(warm_mv[:], 0.0)
            warm_ps = acc_pool.tile([P, 512], dt.float32, tag="acc", name="warm")
            for _ in range(N_WARM):
                nc.tensor.matmul(
                    warm_ps[:], warm_mv[:, 0:P], warm_mv[:], start=True, stop=True
                )

            kv_sb = kv_pool.tile([P, KV_TOT], dt.float16, tag="kvsb")
            wrep = kv_sb[:, 0:KV_WREP]

            def k_ap(s):
                o = KV_WREP + s * KV_REC
                return kv_sb[:, o : o + D]

            def v_ap(s):
                o = KV_WREP + s * KV_REC + D
                return kv_sb[:, o : o + VP]

            # sync ring: packed kv chunks in strip order (head = wrep+s0-3)
            edges = [0] + [KV_WREP + 4 * (c + 1) * KV_REC for c in range(4)]
            for c in range(4):
                sl = slice(edges[c], edges[c + 1])
                nc.sync.dma_start(kv_sb[:, sl], kv_d[:, sl])

            # second warm burst, gated on the kv head DMA via its operands:
            # bridges the PE-idle gap between the dep-free burst and the
            # first real matmul so the HAM activity window never rethrottles
            for _ in range(10):
                nc.tensor.matmul(
                    warm_ps[:], kv_sb[:, 0:P], kv_sb[:, 0:512], start=True, stop=True
                )

            mask16 = kv_pool.tile([P, NT * L], dt.float16, tag="m16")
            m16v = mask16[:].rearrange("p (s i) -> p s i", s=NT)

            # scalar ring: u8 strip groups in consumption order
            mu8 = {}
            for g in U8_GROUPS:
                a, bb = g[0], g[-1]
                t8 = mu8_pool.tile(
                    [P, (bb - a + 1) * L], dt.uint8, tag="mu8", name=f"mu8_{a}"
                )
                nc.scalar.dma_start(t8[:], maskt_d[:, a * L : (bb + 1) * L])
                for s in g:
                    mu8[s] = (t8, (s - a) * L)

            # gpsimd ring: first cast strip immediately; delay the rest with
            # a dummy memset so the kv head isn't diluted at kernel start.
            def cast(s):
                sl = slice(s * L, (s + 1) * L)
                nc.gpsimd.dma_start(mask16[:, sl], maskt_d[:, sl])

            cast(CAST_STRIPS[0])
            delay = const_pool.tile([P, 1024], dt.float32)
            nc.gpsimd.memset(delay[:], 0.0)
            for s in CAST_STRIPS[1:]:
                cast(s)

            # ---- prologue per chunk of 4 strips: sk = k.w ; e = exp(sk) ;
            # ev rows [e*v | e]; converters are independent of this chain.
            sk = small_pool.tile([P, NT], dt.float32, tag="sk")
            e_sb = small_pool.tile([P, NT], dt.float32, tag="e")
            ev = kv_pool.tile([P, NT * VP], dt.float16, tag="ev")
            ev3 = ev[:].rearrange("p (s n) -> p s n", n=VP)

            for c in range(4):
                with tc.tile_wait_until(KV_ARRIVE[c]), tc.high_priority():
                    for s in range(4 * c, 4 * c + 4):
                        junk = junk_pool.tile([P, D], dt.float16, tag="junk")
                        nc.vector.scalar_tensor_tensor(
                            out=junk[:],
                            in0=k_ap(s),
                            scalar=1.0,
                            in1=wrep,
                            op0=mybir.AluOpType.mult,
                            op1=mybir.AluOpType.mult,
                            accum_out=sk[:, s : s + 1],
                        )
                    cs = slice(4 * c, 4 * c + 4)
                    nc.scalar.activation(
                        e_sb[:, cs], sk[:, cs], mybir.ActivationFunctionType.Exp
                    )
                    # scaled moving row [e*v | e] only for this chunk's cast
                    # strip; u8 strips get e fused into the stationary and
                    # use the host-packed [v | 1] moving directly.
                    for s in range(4 * c, 4 * c + 4):
                        if s in CAST_STRIPS:
                            nc.vector.tensor_scalar_mul(
                                ev3[:, s, 0:D], v_ap(s)[:, 0:D], e_sb[:, s : s + 1]
                            )
                            nc.vector.tensor_copy(
                                ev3[:, s : s + 1, D], e_sb[:, s : s + 1]
                            )
                # mask conversions (cast-to-f16 fused with the e_j scale)
                for s in range(4 * c, 4 * c + 4):
                    if s in CAST_STRIPS:
                        continue
                    t8, off = mu8[s]
                    src = t8[:, off : off + L]
                    with tc.tile_wait_until(max(U8_ARRIVE[s], KV_ARRIVE[c] + 0.0015)):
                        if s in DVE_STRIPS:
                            nc.vector.tensor_scalar_mul(
                                mask16[:, s * L : (s + 1) * L], src, e_sb[:, s : s + 1]
                            )
                        else:
                            nc.scalar.mul(
                                mask16[:, s * L : (s + 1) * L], src, e_sb[:, s : s + 1]
                            )

            # ---- two waves of 8 i-tiles; 16 accumulating matmuls each.
            # Wave epilogue (reciprocal + scale + f16 stage) is interleaved
            # with the final matmuls; results stream out per tile-pair on
            # the sync ring during the remaining matmuls.
            for w in range(2):
                accs = []
                for t in range(8 * w, 8 * w + 8):
                    accs.append(
                        acc_pool.tile([P, NE], dt.float32, tag="acc", name=f"acc{t}")
                    )
                outb = out_pool.tile([P, 8 * D], dt.float16, tag="outb", name=f"outb{w}")
                for s in range(NT):
                    mov = ev3[:, s, 0:NE] if s in CAST_STRIPS else v_ap(s)[:, 0:NE]
                    for ti, t in enumerate(range(8 * w, 8 * w + 8)):
                        nc.tensor.matmul(
                            accs[ti][:],
                            m16v[:, s, t * P : (t + 1) * P],
                            mov,
                            start=(s == 0),
                            stop=(s == NT - 1),
                        )
                    # zero-matmuls (+0 into an accumulator, exact no-op)
                    # at supply-stall-prone strip boundaries of wave A:
                    # they execute from the PE FIFO during the stall and
                    # keep the HAM activity window busy, so the stream
                    # resumes at 2.4 GHz instead of re-warming from 1.2
                    if w == 0 and 1 <= s <= 12:
                        for _ in range(3):
                            nc.tensor.matmul(
                                accs[7][:],
                                warm_mv[:, 0:P],
                                warm_mv[:, 0:NE],
                                start=False,
                                stop=False,
                            )
                    for ti, t in enumerate(range(8 * w, 8 * w + 8)):
                        if s == NT - 1:
                            acc = accs[ti]
                            rec = rec_pool.tile([P, 1], dt.float32, tag="rec")
                            nc.vector.reciprocal(rec[:], acc[:, D : D + 1])
                            # alternate the scale between ACT and DVE so
                            # tile pairs drain in parallel (frees psum banks
                            # for the next wave sooner, shortens the tail)
                            if ti % 2 == 0:
                                nc.scalar.mul(
                                    outb[:, ti * D : (ti + 1) * D], acc[:, 0:D], rec[:]
                                )
                            else:
                                nc.vector.tensor_scalar_mul(
                                    outb[:, ti * D : (ti + 1) * D], acc[:, 0:D], rec[:]
                                )
                                # alternate store rings so the final pair
                                # stores drain in parallel, not ring-FIFO
                                eng = nc.sync if (ti // 2) % 2 == 0 else nc.scalar
                                eng.dma_start(
                                    out_d[:, (8 * w + ti - 1) * D : (8 * w + ti + 1) * D],
                                    outb[:, (ti - 1) * D : (ti + 1) * D],
                                )

    nc.compile()
    return nc


def kernel(query, key, value, mask, w_align):
    global LAST_RESULTS
    key = np.asarray(key, dtype=np.float32)
    value = np.asarray(value, dtype=np.float32)
    mask = np.asarray(mask)
    w_align = np.asarray(w_align, dtype=np.float32)

    nc = _build_nc()
    in_maps = []
    for b in range(B):
        # maskt[p, s, t*128+c] = mask[b][i=128t+c, j=128s+p]
        mt = (
            mask[b]
            .astype(np.uint8)
            .reshape(NT, P, NT, P)  # [t, c, s, p]
            .transpose(3, 2, 0, 1)  # [p, s, t, c]
            .reshape(P, NT * L)
        )
        # packed wrep/k/v records, all fp16
        kvb = np.zeros((P, KV_TOT), dtype=np.float16)
        kvb[:, 0:KV_WREP] = w_align[None, :]
        kb = key[b].reshape(NT, P, D).transpose(1, 0, 2)  # [p, s, d]
        vb = value[b].reshape(NT, P, D).transpose(1, 0, 2)
        for s in range(NT):
            o = KV_WREP + s * KV_REC
            kvb[:, o : o + D] = kb[:, s]
            kvb[:, o + D : o + D + D] = vb[:, s]
            kvb[:, o + D + D] = 1.0  # ones col (denominator path headroom)
        in_maps.append({"maskt": np.ascontiguousarray(mt), "kv": kvb})
    try:
        res = run_bass_kernel_spmd(nc, in_maps, core_ids=list(range(B)))
    except Exception:
        # e.g. trace requested but profiling unavailable -- retry untraced
        os.environ["BASS_NEVER_TRACE"] = "1"
        res = run_bass_kernel_spmd(nc, in_maps, core_ids=list(range(B)))
    LAST_RESULTS = res
    out = np.empty((B, L, D), dtype=np.float32)
    for b in range(B):
        ob = res.results[b]["out"].astype(np.float32)  # [p, t*D]
        out[b] = ob.reshape(P, NT, D).transpose(1, 0, 2).reshape(L, D)
    return out


# revision 57
# speedup vs baseline: 1.1073x; 1.0407x over previous
"""Bahdanau additive attention on 8 TRN2 NeuronCores (batch-parallel).

Math: scores[b,i,j] = q[b,i].w + k[b,j].w, masked to -1e9 where mask==0,
softmax over j, then @ value.  The query term q[b,i].w is constant along j,
so it cancels in the softmax:

    out[b,i,:] = (sum_j mask[b,i,j] * e[b,j] * value[b,j,:])
               / (sum_j mask[b,i,j] * e[b,j]),      e[b,j] = exp(k[b,j].w)

(no query needed, no [Lq,Lk] softmax).  Per core: one batch.

Layout strategy: the PE contracts over partitions, so the mask needs j on
partitions.  Rather than transposing on-chip (256 PE transposes ~ 27us),
the host uploads the mask PRE-TRANSPOSED as uint8 in j-major tile order:
maskt[p, s, t*128+c] = mask[i=128t+c, j=128s+p].  That's 4x fewer HBM
bytes than int32 and removes all PE transpose work.  The 0/1 bytes become
fp16 0.0/1.0 stationary operands via three parallel converters that depend
ONLY on the mask bytes (not on the exp chain):
  - SWDGE cast-DMA (u8 -> f16 during the DMA itself, gpsimd ring)
  - DVE tensor_scalar is_gt (u8 in, f16 out)
  - ACT activation-copy (u8 in, f16 out)
The matmul accumulates psum[i, 0:257] = sum_j maskT[j,i] * [e*v | e][j,:]
over 16 j-strips; col 256 gives the softmax denominator.  16 i-tiles run
in two waves of 8 psum banks; epilogue divides and stores fp16, upcast on
the host.

DMA orchestration (the performance-critical part): the 16 SDMA engines
serve all rings round-robin at packet granularity, so a transfer's
completion time tracks the TOTAL dispatched backlog, not its own size;
within one HWDGE ring completions are FIFO.  So: keep total bytes low
(k/wrep in fp16, packed with v into one tensor = 2.1MB; mask u8 4.2MB;
only 4 strips take the 2x-write SWDGE cast path), dispatch in consumption
order per ring, and hold the SWDGE ring back with a dummy gpsimd memset so
the critical kv head isn't diluted at kernel start.  Each [128 x N] HWDGE
dispatch also costs ~0.65us descriptor-generation on its ring, so DMA
count per ring is kept small.

The Tile scheduler's internal DMA-cost model is far too optimistic; left
alone it bakes head-of-line blocking into the engine FIFOs (an op whose
data lands at 22us ordered ahead of ops ready at 14us).  tile_wait_until
annotations carry measured arrival times into the scheduling simulation.

A dependency-free burst of dummy matmuls at kernel start trips the PE HAM
activity monitor to full clock before real work arrives.
"""

import os
import sys
import types

sys.path.insert(0, "/opt/trn_rl_repo")

import numpy as np

import concourse.bacc as bacc
import concourse.tile as tile
from concourse import mybir
from concourse.bass_utils import run_bass_kernel_spmd


def _ensure_ntff_hook_importable():
    """bass_utils imports antenv.axon_hooks when BASS_TRACE is set; this
    image's antenv lacks that module.  Provide it (and register the real
    ctypes NTFF hook if available) so tracing works instead of crashing."""
    if "antenv.axon_hooks" in sys.modules:
        return
    try:
        import antenv
    except ImportError:
        return
    hooks = types.ModuleType("antenv.axon_hooks")
    hooks._hook = None
    hooks.set_axon_ntff_profile_hook = lambda h: setattr(hooks, "_hook", h)
    hooks.get_axon_ntff_profile_hook = lambda: hooks._hook
    sys.modules["antenv.axon_hooks"] = hooks
    antenv.axon_hooks = hooks
    try:
        from trn_agent_boot.trn_boot import _ntff_profile_via_ctypes

        hook = _ntff_profile_via_ctypes("/opt/axon/libaxon_pjrt.so")
        if hook is not None:
            hooks.set_axon_ntff_profile_hook(hook)
    except Exception:
        pass


_ensure_ntff_hook_importable()

P = 128
B = 8
L = 2048
D = 256
NT = L // P  # 16 tiles per dim
NE = D + 1  # 257 = value cols + e col (matmul moving width)
VP = D + 2  # 258 = ev row pitch (even, for engine perf modes)

# packed wrep/k/v record geometry, in fp16 elements per partition
KV_WREP = D  # wrep: 256 f16
KV_REC = D + VP  # per strip: k 256 f16 + v 258 f16
KV_TOT = KV_WREP + NT * KV_REC

# strip -> converter assignment (tunable).  The tail strips are SWDGE
# casts so their readiness is the DMA arrival itself -- no converter-queue
# lag at the end of the supply stream, where it directly sets wave A's end.
CAST_STRIPS = (0, 2, 5, 10, 13, 14, 15)  # SWDGE u8->f16 cast-DMA
DVE_STRIPS = (1, 3, 4, 7, 9, 12)  # u8 load + DVE scaled cast
ACT_STRIPS = (6, 8, 11)  # u8 load + ACT scaled copy
U8_GROUPS = ((1,), (3, 4), (6, 7), (8, 9), (11, 12))
N_WARM = 9

# scheduler hints: realistic data-arrival times (ms) for tile_wait_until,
# measured from HW traces of this exact configuration.
KV_ARRIVE = (0.012, 0.018, 0.023, 0.028)
U8_ARRIVE = {1: 0.0105, 2: 0.013, 3: 0.0165, 4: 0.0165, 6: 0.021, 7: 0.021,
             8: 0.026, 9: 0.026, 11: 0.030, 12: 0.030, 13: 0.028, 14: 0.028}

LAST_RESULTS = None


def _build_nc():
    dt = mybir.dt
    nc = bacc.Bacc("TRN2", target_bir_lowering=False, debug=False, num_devices=B)

    maskt_d = nc.dram_tensor("maskt", [P, NT * L], dt.uint8, kind="ExternalInput").ap()
    kv_d = nc.dram_tensor("kv", [P, KV_TOT], dt.float16, kind="ExternalInput").ap()
    out_d = nc.dram_tensor("out", [P, NT * D], dt.float16, kind="ExternalOutput").ap()

    with tile.TileContext(nc) as tc:
        with (
            tc.tile_pool(name="const", bufs=1) as const_pool,
            tc.tile_pool(name="kv", bufs=1) as kv_pool,
            tc.tile_pool(name="small", bufs=1) as small_pool,
            tc.tile_pool(name="junk", bufs=2) as junk_pool,
            tc.tile_pool(name="mu8", bufs=7) as mu8_pool,
            tc.tile_pool(name="outp", bufs=2) as out_pool,
            tc.tile_pool(name="rec", bufs=8) as rec_pool,
            tc.tile_pool(name="acc", bufs=8, space="PSUM") as acc_pool,
        ):
            # HAM warmup: dummy matmuls with no real dependencies (zeroed
            # data; results never read) to bring the PE to full clock.
            # memset on gpsimd: the vector queue's preamble is longer.
            warm_mv = const_pool.tile([P, 512], dt.float16)
            nc.gpsimd.memset(warm_mv[:], 0.0)
            warm_ps = acc_pool.tile([P, 512], dt.float32, tag="acc", name="warm")
            for _ in range(N_WARM):
                nc.tensor.matmul(
                    warm_ps[:], warm_mv[:, 0:P], warm_mv[:], start=True, stop=True
                )

            kv_sb = kv_pool.tile([P, KV_TOT], dt.float16, tag="kvsb")
            wrep = kv_sb[:, 0:KV_WREP]

            def k_ap(s):
                o = KV_WREP + s * KV_REC
                return kv_sb[:, o : o + D]

            def v_ap(s):
                o = KV_WREP + s * KV_REC + D
                return kv_sb[:, o : o + VP]

            # sync ring: packed kv chunks in strip order (head = wrep+s0-3)
            edges = [0] + [KV_WREP + 4 * (c + 1) * KV_REC for c in range(4)]
            for c in range(4):
                sl = slice(edges[c], edges[c + 1])
                nc.sync.dma_start(kv_sb[:, sl], kv_d[:, sl])

            # second warm burst, gated on the kv head DMA via its operands:
            # bridges the PE-idle gap between the dep-free burst and the
            # first real matmul so the HAM activity window never rethrottles
            for _ in range(10):
                nc.tensor.matmul(
                    warm_ps[:], kv_sb[:, 0:P], kv_sb[:, 0:512], start=True, stop=True
                )

            mask16 = kv_pool.tile([P, NT * L], dt.float16, tag="m16")
            m16v = mask16[:].rearrange("p (s i) -> p s i", s=NT)

            # scalar ring: u8 strip groups in consumption order
            mu8 = {}
            for g in U8_GROUPS:
                a, bb = g[0], g[-1]
                t8 = mu8_pool.tile(
                    [P, (bb - a + 1) * L], dt.uint8, tag="mu8", name=f"mu8_{a}"
                )
                nc.scalar.dma_start(t8[:], maskt_d[:, a * L : (bb + 1) * L])
                for s in g:
                    mu8[s] = (t8, (s - a) * L)

            # gpsimd ring: first cast strip immediately; delay the rest with
            # a dummy memset so the kv head isn't diluted at kernel start.
            def cast(s):
                sl = slice(s * L, (s + 1) * L)
                nc.gpsimd.dma_start(mask16[:, sl], maskt_d[:, sl])

            cast(CAST_STRIPS[0])
            delay = const_pool.tile([P, 1024], dt.float32)
            nc.gpsimd.memset(delay[:], 0.0)
            for s in CAST_STRIPS[1:]:
                cast(s)

            # ---- prologue per chunk of 4 strips: sk = k.w ; e = exp(sk) ;
            # ev rows [e*v | e]; converters are independent of this chain.
            sk = small_pool.tile([P, NT], dt.float32, tag="sk")
            e_sb = small_pool.tile([P, NT], dt.float32, tag="e")
            ev = kv_pool.tile([P, NT * VP], dt.float16, tag="ev")
            ev3 = ev[:].rearrange("p (s n) -> p s n", n=VP)

            for c in range(4):
                with tc.tile_wait_until(KV_ARRIVE[c]), tc.high_priority():
                    for s in range(4 * c, 4 * c + 4):
                        junk = junk_pool.tile([P, D], dt.float16, tag="junk")
                        nc.vector.scalar_tensor_tensor(
                            out=junk[:],
                            in0=k_ap(s),
                            scalar=1.0,
                            in1=wrep,
                            op0=mybir.AluOpType.mult,
                            op1=mybir.AluOpType.mult,
                            accum_out=sk[:, s : s + 1],
                        )
                    cs = slice(4 * c, 4 * c + 4)
                    nc.scalar.activation(
                        e_sb[:, cs], sk[:, cs], mybir.ActivationFunctionType.Exp
                    )
                    # scaled moving row [e*v | e] only for this chunk's cast
                    # strip; u8 strips get e fused into the stationary and
                    # use the host-packed [v | 1] moving directly.
                    for s in range(4 * c, 4 * c + 4):
                        if s in CAST_STRIPS:
                            nc.vector.tensor_scalar_mul(
                                ev3[:, s, 0:D], v_ap(s)[:, 0:D], e_sb[:, s : s + 1]
                            )
                            nc.vector.tensor_copy(
                                ev3[:, s : s + 1, D], e_sb[:, s : s + 1]
                            )
                # mask conversions (cast-to-f16 fused with the e_j scale)
                for s in range(4 * c, 4 * c + 4):
                    if s in CAST_STRIPS:
                        continue
                    t8, off = mu8[s]
                    src = t8[:, off : off + L]
                    with tc.tile_wait_until(max(U8_ARRIVE[s], KV_ARRIVE[c] + 0.0015)):
                        if s in DVE_STRIPS:
                            nc.vector.tensor_scalar_mul(
                                mask16[:, s * L : (s + 1) * L], src, e_sb[:, s : s + 1]
                            )
                        else:
                            nc.scalar.mul(
                                mask16[:, s * L : (s + 1) * L], src, e_sb[:, s : s + 1]
                            )

            # ---- two waves of 8 i-tiles; 16 accumulating matmuls each.
            # Wave epilogue (reciprocal + scale + f16 stage) is interleaved
            # with the final matmuls; results stream out per tile-pair on
            # the sync ring during the remaining matmuls.
            for w in range(2):
                accs = []
                for t in range(8 * w, 8 * w + 8):
                    accs.append(
                        acc_pool.tile([P, NE], dt.float32, tag="acc", name=f"acc{t}")
                    )
                outb = out_pool.tile([P, 8 * D], dt.float16, tag="outb", name=f"outb{w}")
                for s in range(NT):
                    mov = ev3[:, s, 0:NE] if s in CAST_STRIPS else v_ap(s)[:, 0:NE]
                    for ti, t in enumerate(range(8 * w, 8 * w + 8)):
                        nc.tensor.matmul(
                            accs[ti][:],
                            m16v[:, s, t * P : (t + 1) * P],
                            mov,
                            start=(s == 0),
                            stop=(s == NT - 1),
                        )
                    # zero-matmuls (+0 into an accumulator, exact no-op)
                    # at supply-stall-prone strip boundaries of wave A:
                    # they execute from the PE FIFO during the stall and
                    # keep the HAM activity window busy, so the stream
                    # resumes at 2.4 GHz instead of re-warming from 1.2
                    if w == 0 and 1 <= s <= 12:
                        for _ in range(3):
                            nc.tensor.matmul(
                                accs[7][:],
                                warm_mv[:, 0:P],
                                warm_mv[:, 0:NE],
                                start=False,
                                stop=False,
                            )
                # epilogue: all reciprocals first (short DVE ops, issued as
                # each tile's stop-matmul retires), then the scales split
                # DVE/ACT so tile pairs drain in parallel and the last
                # tile's scale isn't serialized behind the recip interleave
                recs = []
                for ti in range(8):
                    rec = rec_pool.tile([P, 1], dt.float32, tag="rec", name=f"r{w}_{ti}")
                    nc.vector.reciprocal(rec[:], accs[ti][:, D : D + 1])
                    recs.append(rec)
                for ti in range(8):
                    acc = accs[ti]
                    if ti % 2 == 0:
                        nc.scalar.mul(
                            outb[:, ti * D : (ti + 1) * D], acc[:, 0:D], recs[ti][:]
                        )
                    else:
                        nc.vector.tensor_scalar_mul(
                            outb[:, ti * D : (ti + 1) * D], acc[:, 0:D], recs[ti][:]
                        )
                        # alternate store rings so the final pair stores
                        # drain in parallel, not ring-FIFO serialized
                        eng = nc.sync if (ti // 2) % 2 == 0 else nc.scalar
                        eng.dma_start(
                            out_d[:, (8 * w + ti - 1) * D : (8 * w + ti + 1) * D],
                            outb[:, (ti - 1) * D : (ti + 1) * D],
                        )

    nc.compile()
    return nc


def kernel(query, key, value, mask, w_align):
    global LAST_RESULTS
    key = np.asarray(key, dtype=np.float32)
    value = np.asarray(value, dtype=np.float32)
    mask = np.asarray(mask)
    w_align = np.asarray(w_align, dtype=np.float32)

    nc = _build_nc()
    in_maps = []
    for b in range(B):
        # maskt[p, s, t*128+c] = mask[b][i=128t+c, j=128s+p]
        mt = (
            mask[b]
            .astype(np.uint8)
            .reshape(NT, P, NT, P)  # [t, c, s, p]
            .transpose(3, 2, 0, 1)  # [p, s, t, c]
            .reshape(P, NT * L)
        )
        # packed wrep/k/v records, all fp16
        kvb = np.zeros((P, KV_TOT), dtype=np.float16)
        kvb[:, 0:KV_WREP] = w_align[None, :]
        kb = key[b].reshape(NT, P, D).transpose(1, 0, 2)  # [p, s, d]
        vb = value[b].reshape(NT, P, D).transpose(1, 0, 2)
        for s in range(NT):
            o = KV_WREP + s * KV_REC
            kvb[:, o : o + D] = kb[:, s]
            kvb[:, o + D : o + D + D] = vb[:, s]
            kvb[:, o + D + D] = 1.0  # ones col (denominator path headroom)
        in_maps.append({"maskt": np.ascontiguousarray(mt), "kv": kvb})
    try:
        res = run_bass_kernel_spmd(nc, in_maps, core_ids=list(range(B)))
    except Exception:
        # e.g. trace requested but profiling unavailable -- retry untraced
        os.environ["BASS_NEVER_TRACE"] = "1"
        res = run_bass_kernel_spmd(nc, in_maps, core_ids=list(range(B)))
    LAST_RESULTS = res
    out = np.empty((B, L, D), dtype=np.float32)
    for b in range(B):
        ob = res.results[b]["out"].astype(np.float32)  # [p, t*D]
        out[b] = ob.reshape(P, NT, D).transpose(1, 0, 2).reshape(L, D)
    return out
